# revision 1
# baseline (speedup 1.0000x reference)
"""ADDS loss kernel for Trainium2, SPMD over 8 NeuronCores.

Problem: pred = model_points @ pred_R^T + pred_t (per batch), gt likewise;
d2[b,n,m] = ||pred[b,n] - gt[b,m]||^2; out = mean_{b,n} sqrt(max(min_m d2, 0)).

Sharding: data-parallel over batch B=32 -> 4 batches per core, one 5-row
operand group per batch at partition base 32*b:
  pred_stuff rows = [-2*p_x, -2*p_y, -2*p_z, pn2, 1]
  gt_stuff   rows = [g_x, g_y, g_z, 1, gn2]
so a K=5 matmul yields d2[n, m] = -2 p.g + pn2[n] + gn2[m] directly in PSUM.

Reduction: per (n_chunk, batch) group the 2048 m-values sit in PSUM (two
[128,1024] tiles). Most groups: ScalarE converts them to bf16 SBUF and
VectorE runs a 2x-mode TT-min tree + final reduce; 1-in-7 groups VectorE
reduce_mins straight from PSUM (ACT/DVE load balance). Mins are clamped,
sqrt'd (batched) and summed per partition; the host sums the 8x[128,1]
per-core partials and divides by B*N.

Main matmuls use float32r (~12-bit mantissa, full-rate streaming); host
pre-rounds the inputs to f32r precision. Overall rel err vs the fp32
reference is ~1e-5, ~1000x inside the 2e-2 gate.
"""

import numpy as np

import concourse.bacc as bacc_mod
import concourse.mybir as mybir
from concourse.tile import TileContext
from concourse.bass_utils import run_bass_kernel_spmd

B = 32
N = 2048
NCORES = 8
BPC = B // NCORES  # batches per core = 4
FP32 = mybir.dt.float32
BF16 = mybir.dt.bfloat16
AF = mybir.ActivationFunctionType
OP = mybir.AluOpType

# tuning knobs (overridable per-build)
DEFAULT_CFG = dict(
    bias_dve_p=True,   # pred-side phase-A bias copies on DVE (ACT/DVE balance)
    preload_sqrt=True,  # dummy early sqrt pulls the ACT table load into the ramp
    tail_direct=False,  # tail-direct placement measured neutral; keep off
    split_sqrt=False,   # split sqrt measured neutral; keep off
    interleave_a=True,   # phase A: alternate gt/pred per chunk (faster ramp)
    halftree=True,       # tree level 1 per converted half (earlier DVE start)
    tree_depth=2,        # TT-min tree levels after the half-level
    warmup=0,            # PE warm-up bursts never helped; keep 0
    wide=False,          # two [128,1024] psum tiles/group beat one [128,2048]
    nd_mod=5,            # 1-in-5 groups DVE-direct (ACT/DVE balance)
    sbf_bufs=3,
    work_bufs=2,
)


def build_kernel(**cfg_over):
    cfg = dict(DEFAULT_CFG)
    cfg.update(cfg_over)
    nc = bacc_mod.Bacc()

    F32R = mybir.dt.float32r
    KF = 27
    pointsT_ext = nc.declare_dram_parameter("pointsT", [KF, N], F32R, isOutput=False)
    Rp_ext = nc.declare_dram_parameter("Rp", [KF, 128], F32R, isOutput=False)
    Rg_ext = nc.declare_dram_parameter("Rg", [KF, 128], F32R, isOutput=False)
    biasp_ext = nc.declare_dram_parameter("biasp", [128, 1], FP32, isOutput=False)
    biasg_ext = nc.declare_dram_parameter("biasg", [128, 1], FP32, isOutput=False)
    out_ext = nc.declare_dram_parameter("out", [128, 1], FP32, isOutput=True)

    with TileContext(nc) as tc:
        with (
            tc.tile_pool(name="persist", bufs=1) as persist,
            tc.tile_pool(name="work", bufs=cfg["work_bufs"]) as work,
            tc.tile_pool(name="sbf", bufs=cfg["sbf_bufs"]) as sbf,
            tc.tile_pool(name="ps", bufs=(2 if cfg["wide"] else 4), space="PSUM") as ps,
        ):
            # ---- load inputs ----
            def load(ext, shape, nm, dt=FP32):
                t = persist.tile(shape, dt, tag=nm, name=nm)
                nc.sync.dma_start(out=t[:, :], in_=ext[:, :])
                return t

            F32R = mybir.dt.float32r
            pointsT = load(pointsT_ext, [KF, N], "pointsT_sb", F32R)
            Rsb = {}
            biassb = {}
            for side, (R_ext, b_ext) in (
                ("p", (Rp_ext, biasp_ext)),
                ("g", (Rg_ext, biasg_ext)),
            ):
                Rsb[side] = load(R_ext, [KF, 128], f"R{side}_sb", F32R)
                biassb[side] = load(b_ext, [128, 1], f"bias{side}_sb")

            # Preload the sqrt activation-table set FIRST in ACT's stream:
            # the ~2.7us ACT_TABLE_LOAD then overlaps the input DMAs instead
            # of stalling mid-ramp bias copies or the final sqrt in the tail.
            # (Identity/Copy are filler funcs present in every table set.)
            roots2 = persist.tile([128, 16 * BPC], FP32, tag="roots2", name="roots2")
            if cfg["preload_sqrt"]:
                nc.scalar.activation(
                    roots2[0:1, 0:1], biassb["p"][0:1, :], AF.Sqrt
                )

            # ---- Phase A: build stuff_p / stuff_g (all f32r) ----
            # Inputs arrive pre-rounded to f32r precision from the host.
            # gt side first: phase B needs all gt m-chunks but only the
            # first pred n-chunks to start.
            stuff = {}
            stps = {}
            for side in ("g", "p"):
                stps[side] = persist.tile(
                    [128, N], F32R, tag=f"stp{side}", name=f"stp{side}_sb"
                )
                stuff[side] = stps[side]
            if cfg["interleave_a"]:
                order = [
                    (side, c)
                    for c in range(N // 512)
                    for side in ("g", "p")
                ]
            else:
                order = [
                    (side, c)
                    for side in ("g", "p")
                    for c in range(N // 512)
                ]
            for side, c in order:
                stp = stps[side]
                cs = slice(c * 512, (c + 1) * 512)
                # One K=9 matmul over [x, x^2, xy] features emits the coord
                # rows AND the norm row (host folded -2R / R^T R / 2R^T t
                # into the weights; t / t^T t / 1 come via the bias vector).
                T = ps.tile([128, 512], FP32, tag="psb", name="psb")
                nc.tensor.matmul(
                    T[:, :], Rsb[side][:, :], pointsT[:, cs],
                    start=True, stop=True,
                )
                if cfg["bias_dve_p"] and side == "p":
                    nc.vector.tensor_scalar(
                        stp[:, cs], T[:, :], biassb[side][:, :], None, op0=OP.add
                    )
                else:
                    nc.scalar.activation(
                        stp[:, cs], T[:, :], AF.Identity,
                        bias=biassb[side][:, :], scale=1.0,
                    )

            # ---- PE warm-up: a dense burst of junk matmuls keeps the
            # HAM activity monitor busy so the PE clock ramps to 2.4 GHz
            # before (and into) the main loop. Uses one pooled PSUM slot,
            # released after a single cheap consume.
            if cfg["warmup"]:
                wtile = ps.tile([128, 512], FP32, tag="psb", name="warmtile")
                for _w in range(cfg["warmup"]):
                    nc.tensor.matmul(
                        wtile[:, :],
                        stuff["p"][0:5, 0:128],
                        stuff["g"][0:5, 0:512],
                        start=True,
                        stop=True,
                    )
                wres = persist.tile([128, 1], FP32, tag="wres", name="wres")
                nc.vector.tensor_reduce(
                    wres[:, :], wtile[:, 0:64], axis=mybir.AxisListType.X, op=OP.min
                )
                wjunk = nc.dram_tensor("warm_junk", [128, 1], FP32)
                nc.sync.dma_start(out=wjunk[:, :], in_=wres[:, :])

            # ---- Phase B: main loop ----
            # Per (nch, b) group: 2048 m-values in PSUM ([128,2048] as one
            # tile, or two [128,1024] tiles), 4 f32r matmuls. Tree groups:
            # ACT bf16-converts to S, DVE runs a 2x TT-min tree; direct
            # groups (1 in nd_mod): DVE reduce_min straight from PSUM.
            # Clamped mins collect into roots; sqrt batched at the end.
            roots = persist.tile([128, 16 * BPC], FP32, tag="roots", name="roots")
            for nch in range(16):
                min4 = work.tile([128, BPC], FP32, tag="min4", name="min4")
                for b in range(BPC):
                    g = nch * BPC + b
                    lhs = stuff["p"][32 * b : 32 * b + 5, nch * 128 : (nch + 1) * 128]
                    if cfg["tail_direct"]:
                        direct = (g % cfg["nd_mod"] == cfg["nd_mod"] - 1 and g < 60) or g >= 62
                    else:
                        direct = (g % cfg["nd_mod"]) == (cfg["nd_mod"] - 1)
                    if cfg["wide"]:
                        P = ps.tile([128, 2048], FP32, tag="psb", name="psb")
                        halves = [P[:, 0:1024], P[:, 1024:2048]]
                        for mc in range(4):
                            nc.tensor.matmul(
                                P[:, mc * 512 : (mc + 1) * 512],
                                lhs,
                                stuff["g"][32 * b : 32 * b + 5, mc * 512 : (mc + 1) * 512],
                                start=True,
                                stop=True,
                                tile_position=(32 * b, 0),
                            )
                        wholes = [P[:, :]]
                    else:
                        halves = []
                        for h in range(2):
                            P = ps.tile([128, 1024], FP32, tag="psb", name="psb")
                            halves.append(P[:, :])
                            for mc in range(2):
                                m0 = (2 * h + mc) * 512
                                nc.tensor.matmul(
                                    P[:, mc * 512 : (mc + 1) * 512],
                                    lhs,
                                    stuff["g"][32 * b : 32 * b + 5, m0 : m0 + 512],
                                    start=True,
                                    stop=True,
                                    tile_position=(32 * b, 0),
                                )
                        wholes = None
                    if direct:
                        if cfg["wide"]:
                            nc.vector.tensor_reduce(
                                min4[:, b : b + 1], wholes[0],
                                axis=mybir.AxisListType.X, op=OP.min,
                            )
                        else:
                            m2 = work.tile([128, 2], FP32, tag="m2", name="m2")
                            for h in range(2):
                                nc.vector.tensor_reduce(
                                    m2[:, h : h + 1], halves[h],
                                    axis=mybir.AxisListType.X, op=OP.min,
                                )
                            nc.vector.tensor_reduce(
                                min4[:, b : b + 1], m2[:, :],
                                axis=mybir.AxisListType.X, op=OP.min,
                            )
                    else:
                        S = sbf.tile([128, 2048], BF16, tag="S", name="S")
                        if cfg["wide"]:
                            nc.scalar.copy(S[:, :], wholes[0])
                        else:
                            for h in range(2):
                                nc.scalar.copy(
                                    S[:, h * 1024 : (h + 1) * 1024], halves[h]
                                )
                        if cfg["halftree"]:
                            # level 1 per copied half: DVE starts after the
                            # first ACT copy instead of both
                            u1a = sbf.tile([128, 512], BF16, tag="u1a", name="u1a")
                            nc.vector.tensor_tensor(
                                u1a[:, :], S[:, 0:512], S[:, 512:1024], op=OP.min
                            )
                            u1b = sbf.tile([128, 512], BF16, tag="u1b", name="u1b")
                            nc.vector.tensor_tensor(
                                u1b[:, :], S[:, 1024:1536], S[:, 1536:2048], op=OP.min
                            )
                            u1 = sbf.tile([128, 512], BF16, tag="u1", name="u1")
                            nc.vector.tensor_tensor(
                                u1[:, :], u1a[:, :], u1b[:, :], op=OP.min
                            )
                            last = u1
                            width = 256
                        else:
                            u1 = sbf.tile([128, 1024], BF16, tag="u1", name="u1")
                            nc.vector.tensor_tensor(
                                u1[:, :], S[:, 0:1024], S[:, 1024:2048], op=OP.min
                            )
                            last = u1
                            width = 512
                        for lvl in range(cfg["tree_depth"] - 1):
                            nxt = sbf.tile(
                                [128, width], BF16, tag=f"u{lvl+2}", name=f"u{lvl+2}"
                            )
                            nc.vector.tensor_tensor(
                                nxt[:, :], last[:, 0:width], last[:, width : 2 * width],
                                op=OP.min,
                            )
                            last = nxt
                            width //= 2
                        nc.vector.tensor_reduce(
                            min4[:, b : b + 1], last[:, :],
                            axis=mybir.AxisListType.X, op=OP.min,
                        )
                # clamp at 0 into roots (sqrt batched at the end)
                nc.vector.tensor_scalar(
                    roots[:, nch * BPC : (nch + 1) * BPC], min4[:, :], 0.0, None,
                    op0=OP.max,
                )

            # ---- final: sqrt then sum over the 64 roots columns ----
            if cfg["split_sqrt"]:
                nc.scalar.activation(roots2[:, 0:32], roots[:, 0:32], AF.Sqrt)
                nc.scalar.activation(roots2[:, 32:64], roots[:, 32:64], AF.Sqrt)
            else:
                nc.scalar.activation(roots2[:, :], roots[:, :], AF.Sqrt)
            acc = persist.tile([128, 1], FP32, tag="acc", name="acc")
            nc.vector.tensor_reduce(
                acc[:, :], roots2[:, :], axis=mybir.AxisListType.X, op=OP.add
            )
            nc.sync.dma_start(out=out_ext[:, :], in_=acc[:, :])

    nc.compile()
    return nc


_NC_CACHE = None


def _get_nc():
    global _NC_CACHE
    if _NC_CACHE is None:
        _NC_CACHE = build_kernel()
    return _NC_CACHE


def _round_f32r(x):
    """Round fp32 to float32r precision (12-bit mantissa, round-to-nearest)."""
    xi = np.ascontiguousarray(x, np.float32).view(np.uint32)
    drop = 11
    bias = ((xi >> drop) & 1) + ((1 << (drop - 1)) - 1)
    mask = np.uint32(0xFFFFFFFF ^ ((1 << drop) - 1))
    return ((xi + bias) & mask).view(np.float32)


def make_in_maps(pred_R, pred_t, gt_R, gt_t, model_points):
    # point features: rows [x, y, z, xx, yy, zz, xy, xz, yz], then the same
    # 9 rows again (paired with coeff residuals), then the features' own
    # f32r residuals (paired with hi coeffs) -> first-order error
    # compensation of the f32r transform at zero matmul cost (K 9->27).
    x = model_points.T.astype(np.float32)  # [3, N]
    feats = np.concatenate(
        [
            x,
            x * x,
            np.stack([x[0] * x[1], x[0] * x[2], x[1] * x[2]]),
        ],
        axis=0,
    )  # [9, N]
    fh = _round_f32r(np.ascontiguousarray(feats))
    fl = _round_f32r(feats - fh)
    pointsT = np.concatenate([fh, fh, fl], axis=0)  # [27, N]
    in_maps = []
    for core in range(NCORES):
        Rp = np.zeros((27, 128), np.float32)
        Rg = np.zeros((27, 128), np.float32)
        biasp = np.zeros((128, 1), np.float32)
        biasg = np.zeros((128, 1), np.float32)
        for b in range(BPC):
            gb = core * BPC + b
            base = 32 * b
            for R, t, Rm, biasm, scale, normrow, onesrow in (
                (pred_R[gb], pred_t[gb], Rp, biasp, -2.0, 3, 4),
                (gt_R[gb], gt_t[gb], Rg, biasg, 1.0, 4, 3),
            ):
                # exact coefficient vectors over the 9 features
                coord = np.zeros((9, 3), np.float32)
                coord[0:3, :] = scale * R.T
                RtR = (R.T @ R).astype(np.float32)
                norm = np.zeros(9, np.float32)
                norm[0:3] = 2.0 * (R.T @ t)
                norm[3:6] = np.diag(RtR)
                norm[6:9] = 2.0 * np.array([RtR[0, 1], RtR[0, 2], RtR[1, 2]])
                # hi coeffs pair with feat rows 0:9 and feat residuals 18:27;
                # coeff residuals pair with the duplicated feat rows 9:18
                ch_c = _round_f32r(coord)
                Rm[0:9, base : base + 3] = ch_c
                Rm[9:18, base : base + 3] = _round_f32r(coord - ch_c)
                Rm[18:27, base : base + 3] = ch_c
                ch_n = _round_f32r(norm)
                Rm[0:9, base + normrow] = ch_n
                Rm[9:18, base + normrow] = _round_f32r(norm - ch_n)
                Rm[18:27, base + normrow] = ch_n
                biasm[base : base + 3, 0] = scale * t
                biasm[base + normrow, 0] = float(t @ t)
                # ones row via bias
                biasm[base + onesrow, 0] = 1.0
        in_maps.append(
            {
                "pointsT": pointsT,
                "Rp": Rp,
                "Rg": Rg,
                "biasp": biasp,
                "biasg": biasg,
            }
        )
    return in_maps


def kernel(pred_R, pred_t, gt_R, gt_t, model_points):
    pred_R = np.asarray(pred_R, np.float32)
    pred_t = np.asarray(pred_t, np.float32)
    gt_R = np.asarray(gt_R, np.float32)
    gt_t = np.asarray(gt_t, np.float32)
    model_points = np.asarray(model_points, np.float32)

    nc = _get_nc()
    in_maps = make_in_maps(pred_R, pred_t, gt_R, gt_t, model_points)
    last_err = None
    for wait_s in (5, 15, 30, 45, 0):
        try:
            res = run_bass_kernel_spmd(nc, in_maps, core_ids=list(range(NCORES)))
            break
        except Exception as e:  # transient device faults recover on retry
            last_err = e
            if wait_s == 0:
                raise
            import time as _time

            _time.sleep(wait_s)
    else:
        raise last_err
    total = np.float64(0.0)
    for r in res.results:
        total += np.asarray(r["out"], np.float64).sum()
    return np.float32(total / (B * N))

